# revision 33
# baseline (speedup 1.0000x reference)
"""Trainium2 Bass kernel for CausalSelfAttention with block-repeated causal mask.

Problem: B=2, T=3072, C=1024, H=16 heads, d=64.
  q/k/v = x @ W{q,k,v}.T + b;  scores = q k^T / 8, masked by
  (i % 1024) >= (j % 1024) (tril(1024) tiled 3x3), softmax, y = attn @ v,
  out = y @ Wp.T + bp.

Sharding (8 cores): core i handles batch b = i//4 and heads 4*(i%4)..4*(i%4)+3
(data parallel on B, tensor parallel on heads).  Each core computes a partial
output projection (its 4 heads' contribution, no bias); the host sums the 4
partials per batch and adds bp (the TP all-reduce done at unshard time).

Device schedule (v2): query tile-pairs processed in order of increasing
block-row (ri1 = 1,3,5,7); k/v projections are computed incrementally for the
newly needed key-tile rows and interleaved as PE filler between score groups,
so the ACT engine (exp softmax, the co-bottleneck) is fed from ~10us onward
instead of idling behind an 80us projection prologue.  xT streams in per
group.

Per (query-pair, head-pair), key tiles go in groups of 2 into a [128,1024]
score psum laid out h2*512 + p*128 with p = (j0qi0, j0qi1, j1qi1, j1qi0),
so the fully-masked (j1,qi0) combination of diagonal groups is neither
scored nor exp'd (ACT reads a [128,2,384] view); the non-diagonal j1 is a
single n=256 matmul streaming qi1-then-qi0 through a reversed rhs AP.
Scores are K=64 matmuls (kT/qT in a [128=pair-channels, pair, T] bf16
layout, tile_position row 64*h2; consecutive matmuls alternate psum banks).
exp on ACT (scale=1/8 folded), diagonal tile masks multiplied on DVE,
attn@v lags one group behind exp and accumulates [q, (qi-bank, h2)] with a
ones column for rowsums, reciprocal-normalize on DVE, DMA-transpose XBAR
to [d, q] (PE transpose for the final pair's serial tail), output
projection per query tile.  Tails (transpose, outproj) are deferred through
the same work queue as the k/v filler chunks, one item per score group, so
the in-order PE never waits on the DVE normalize chain.
"""

import numpy as np
import ml_dtypes

import concourse.bass as bass
from concourse import bacc
import concourse.mybir as mybir
from concourse.bass import ts
from concourse.tile import TileContext
from concourse.bass_utils import run_bass_kernel_spmd
from concourse.masks import make_identity, make_upper_triangular

B, T, C, H = 2, 3072, 1024, 16
D = 64                  # head dim
NCORE = 8
HPC = 4                 # heads per core
PAIRS = 2               # head pairs per core
CHS = HPC * D           # 256 channels per core
NKT = T // 128          # 24 key tiles
RPB = 8                 # 128-tiles per 1024 mask block
DE = D + 1              # head value cols incl. ones column
NCI = C // 128          # 8 contraction tiles

BF16 = mybir.dt.bfloat16
F32 = mybir.dt.float32

_CACHE = {}


def _build():
    nc = bacc.Bacc()

    xTd = nc.dram_tensor("xT", [C, T], BF16, kind="ExternalInput")
    wqkvd = nc.dram_tensor("wqkv", [C, 3 * CHS], BF16, kind="ExternalInput")
    wpd = nc.dram_tensor("wpT", [CHS, C], BF16, kind="ExternalInput")
    bqkd = nc.dram_tensor("bqk", [128, 2 * PAIRS], F32, kind="ExternalInput")
    bvd = nc.dram_tensor("bv", [128, CHS], F32, kind="ExternalInput")
    outd = nc.dram_tensor("out", [T, C], F32, kind="ExternalOutput")

    with TileContext(nc) as tc:
        with (
            tc.tile_pool(name="const", bufs=1) as const,
            tc.tile_pool(name="exps", bufs=8) as expp,
            tc.tile_pool(name="ynorm", bufs=8) as ynp,
            tc.tile_pool(name="ytp", bufs=6) as ytp,
            tc.tile_pool(name="outp", bufs=3) as outp,
            tc.tile_pool(name="small", bufs=16) as smallp,
            tc.tile_pool(name="ps_s", bufs=2, space="PSUM") as ps_s,
            tc.tile_pool(name="ps_y", bufs=1, space="PSUM") as ps_y,
            tc.tile_pool(name="ps_o", bufs=2, space="PSUM") as ps_o,
        ):
            # ---------------- constants / weights into SBUF ----------------
            xT_sb = const.tile([128, NCI, T], BF16)
            xT_r = xTd[:, :].rearrange("(a p) (blk t) -> a p blk t", p=128, blk=3)

            def dma_xT_group(g):
                # tiles r in (2g, 2g+1) for all 3 blocks: 256 cols per block
                for a in range(NCI):
                    dst = xT_sb[:, a, :].rearrange("p (blk t) -> p blk t", blk=3)
                    nc.sync.dma_start(
                        out=dst[:, :, 2 * g * 128 : (2 * g + 2) * 128],
                        in_=xT_r[a][:, :, 2 * g * 128 : (2 * g + 2) * 128],
                    )

            # weights + biases ride the idle gpsimd DMA queue so the sync
            # queue's serial descriptor issue doesn't delay xT / outputs
            wqkv_sb = const.tile([128, NCI, 3 * CHS], BF16)
            wqkv_r = wqkvd[:, :].rearrange("(a p) c -> a p c", p=128)
            for a in range(NCI):  # wk section first: k-chunks gate the ramp
                nc.gpsimd.dma_start(
                    out=wqkv_sb[:, a, CHS : 2 * CHS], in_=wqkv_r[a][:, CHS : 2 * CHS]
                )
            for a in range(NCI):
                nc.gpsimd.dma_start(
                    out=wqkv_sb[:, a, 0:CHS], in_=wqkv_r[a][:, 0:CHS]
                )
                nc.gpsimd.dma_start(
                    out=wqkv_sb[:, a, 2 * CHS :], in_=wqkv_r[a][:, 2 * CHS :]
                )
            bqk_ld = const.tile([128, 2 * PAIRS], F32)
            bv_ld = const.tile([128, CHS], F32)
            nc.gpsimd.dma_start(out=bqk_ld, in_=bqkd[:, :])
            nc.gpsimd.dma_start(out=bv_ld, in_=bvd[:, :])
            wp_sb = const.tile([128, PAIRS, C], BF16)
            for a in range(PAIRS):
                nc.gpsimd.dma_start(
                    out=wp_sb[:, a, :],
                    in_=wpd[:, :].rearrange("(a p) c -> a p c", p=128)[a],
                )
            # DVE-local copies: consumers then never need a DMA sem wait
            bqk_sb = const.tile([128, 2 * PAIRS], F32)
            bv_sb = const.tile([128, CHS], F32)
            nc.vector.tensor_copy(bqk_sb, bqk_ld)
            nc.vector.tensor_copy(bv_sb, bv_ld)
            bv_r = bv_sb.rearrange("p (h e) -> p h e", e=D)

            ident = const.tile([128, 128], BF16)
            make_identity(nc, ident)
            # mask[k', q'] = 1 where q' >= k' (keep), else 0
            mask_sb = const.tile([128, 128], BF16)
            make_upper_triangular(nc, mask_sb, val=1.0, diag=True)

            # q/k in pair-channel layout [128 = (h_even 0:64 | h_odd 64:128), pair, t]
            qT_sb = const.tile([128, PAIRS, T], BF16)
            kT_sb = const.tile([128, PAIRS, T], BF16)
            v_sb = const.tile([128, NKT, HPC * DE], BF16)
            v_ones = v_sb.rearrange("p j (h e) -> p j h e", e=DE)[:, :, :, D : D + 1]
            nc.vector.memset(v_ones, 1.0)

            # ---------------- projection emitters ----------------
            def emit_k_chunk(pr, rr, pool=None):
                # kT pair pr for tile-row rr (=2g or 2g+1), all 3 blocks
                if pool is None:
                    pk = ps_o.tile([128, 512], F32, name="pk", tag="po")
                else:
                    pk = pool.tile([128, 512], F32, name="pscore", tag="sc")
                for ci in range(NCI):
                    rhs = xT_sb[:, ci, :].rearrange("p (blk t) -> p blk t", blk=3)
                    nc.tensor.matmul(
                        pk[:, 0:384],
                        lhsT=wqkv_sb[:, ci, CHS + pr * 128 : CHS + (pr + 1) * 128],
                        rhs=rhs[:, :, rr * 128 : (rr + 1) * 128],
                        start=(ci == 0),
                        stop=(ci == NCI - 1),
                    )
                dst = kT_sb[:, pr, :].rearrange("p (blk t) -> p blk t", blk=3)
                nc.vector.tensor_add(
                    dst[:, :, rr * 128 : (rr + 1) * 128],
                    pk[:, 0:384].rearrange("p (blk t) -> p blk t", blk=3),
                    bqk_sb[:, PAIRS + pr : PAIRS + pr + 1].to_broadcast((128, 3, 128)),
                )

            def emit_v_chunk(g, blk, pool=None):
                J0 = blk * RPB + 2 * g
                if pool is None:
                    pv = ps_o.tile([128, 512], F32, name="pv", tag="po")
                else:
                    pv = pool.tile([128, 512], F32, name="pscore", tag="sc")
                for sub in range(2):
                    tt = J0 + sub
                    for ci in range(NCI):
                        nc.tensor.matmul(
                            pv[:, ts(sub, 256)],
                            lhsT=xT_sb[:, ci, ts(tt, 128)],
                            rhs=wqkv_sb[:, ci, 2 * CHS : 3 * CHS],
                            start=(ci == 0),
                            stop=(ci == NCI - 1),
                        )
                for sub in range(2):
                    tt = J0 + sub
                    vt = v_sb[:, tt, :].rearrange("p (h e) -> p h e", e=DE)[:, :, 0:D]
                    pvr = pv[:, ts(sub, 256)].rearrange("p (h e) -> p h e", e=D)
                    nc.vector.tensor_add(vt, pvr, bv_r)

            def emit_q_proj(qp):
                c0 = 2 * qp * 128
                for pr in range(PAIRS):
                    pq = ps_o.tile([128, 512], F32, name="pq", tag="po")
                    for ci in range(NCI):
                        nc.tensor.matmul(
                            pq[:, 0:256],
                            lhsT=wqkv_sb[:, ci, ts(pr, 128)],
                            rhs=xT_sb[:, ci, c0 : c0 + 256],
                            start=(ci == 0),
                            stop=(ci == NCI - 1),
                        )
                    nc.vector.tensor_add(
                        qT_sb[:, pr, c0 : c0 + 256],
                        pq[:, 0:256],
                        bqk_sb[:, pr : pr + 1].to_broadcast((128, 256)),
                    )

            # ---------------- attention per query tile-pair ----------------
            def emit_attention(qp, work, last=False):
                ri0 = (2 * qp) % RPB
                ri1 = ri0 + 1
                q0 = 2 * qp
                allowed = [b * RPB + r for b in range(3) for r in range(ri1 + 1)]
                allowed_q = [
                    [j for j in allowed if j % RPB <= ri0],
                    allowed,
                ]
                groups = [allowed[i : i + 2] for i in range(0, len(allowed), 2)]

                it = 0
                yts = []
                for hp in range(PAIRS):
                    # [q, (qi-bank, h2)] psum: qi0 cols 0:130 (bank0), qi1
                    # cols 512:642 (bank1) so qi-inner attn@v alternates banks
                    py = ps_y.tile([128, 1024], F32, name="py", tag="py")

                    def emit_attnv(g, esb, py=py, allowed=allowed, allowed_q=allowed_q, hp=hp, ri1=ri1):
                        diag = g[1] % RPB == ri1
                        for j, J in enumerate(g):
                            for h2 in range(2):
                                hg = hp * 2 + h2
                                for qi in range(2):
                                    if j == 1 and qi == 0 and diag:
                                        continue
                                    p = 2 * j + (qi if j == 0 else 1 - qi)
                                    nc.tensor.matmul(
                                        py[:, qi * 512 + h2 * DE : qi * 512 + (h2 + 1) * DE],
                                        lhsT=esb[
                                            :,
                                            h2 * 512 + p * 128 : h2 * 512 + p * 128 + 128,
                                        ],
                                        rhs=v_sb[:, J, hg * DE : (hg + 1) * DE],
                                        start=(J == allowed[0] and h2 == 0),
                                        stop=(J == allowed_q[qi][-1]),
                                        skip_group_check=True,
                                    )

                    prev = None  # attn@v lags one group: never waits on exp
                    for gi, g in enumerate(groups):
                        diag = g[1] % RPB == ri1
                        pscore = ps_s.tile([128, 1024], F32, name="pscore", tag="sc")
                        # col layout per h2: p*128 with p = (j0qi0, j0qi1,
                        # j1qi1, j1qi0) so the diagonal groups' unscored
                        # (j1,qi0) block sits at the end of the exp view.
                        # j1 streams qi1-then-qi0 via a qi-reversed rhs AP.
                        rq = qT_sb[:, hp, q0 * 128 : q0 * 128 + 256]
                        rq_rev = rq.rearrange("p (a c) -> p a c", a=2)[:, ::-1, :]
                        for h2 in range(2):
                            nc.tensor.matmul(
                                pscore[:, h2 * 512 : h2 * 512 + 256],
                                lhsT=kT_sb[64 * h2 : 64 * h2 + 64, hp, ts(g[0], 128)],
                                rhs=rq[64 * h2 : 64 * h2 + 64, :],
                                start=True,
                                stop=True,
                                tile_position=(64 * h2, 0),
                                skip_group_check=True,
                            )
                        for h2 in range(2):
                            nc.tensor.matmul(
                                pscore[:, h2 * 512 + 256 : h2 * 512 + 512],
                                lhsT=kT_sb[64 * h2 : 64 * h2 + 64, hp, ts(g[1], 128)],
                                rhs=rq_rev[64 * h2 : 64 * h2 + 64, :, :],
                                start=False,
                                stop=True,
                                tile_position=(64 * h2, 0),
                                skip_group_check=True,
                            )
                        esb = expp.tile([128, 1024], BF16)
                        if diag:
                            nc.scalar.activation(
                                esb.rearrange("p (h c) -> p h c", h=2)[:, :, 0:384],
                                pscore.rearrange("p (h c) -> p h c", h=2)[:, :, 0:384],
                                mybir.ActivationFunctionType.Exp,
                                scale=0.125,
                            )
                            # mask the two diagonal tiles: (j0,qi0)=p0, (j1,qi1)=p2
                            for h2 in range(2):
                                for p in (0, 2):
                                    sl = esb[:, h2 * 512 + p * 128 : h2 * 512 + p * 128 + 128]
                                    nc.vector.tensor_mul(sl, sl, mask_sb)
                        else:
                            nc.scalar.activation(
                                esb, pscore, mybir.ActivationFunctionType.Exp, scale=0.125
                            )
                        it += 1
                        if work:
                            work.pop(0)[1]()
                        if last:
                            emit_attnv(g, esb)
                        else:
                            if prev is not None:
                                emit_attnv(*prev)
                            prev = (g, esb)
                    if not last:
                        emit_attnv(*prev)

                    # normalize on DVE immediately (frees py for the next
                    # head-pair without blocking PE); defer the PE transposes
                    yns = []
                    for qi in range(2):
                        yn = ynp.tile([128, 2, D], BF16, name="yn")
                        for h2 in range(2):
                            rc = smallp.tile([128, 1], F32)
                            nc.vector.reciprocal(
                                rc, py[:, qi * 512 + h2 * DE + D : qi * 512 + h2 * DE + DE]
                            )
                            nc.vector.tensor_scalar_mul(
                                yn[:, h2, :], py[:, qi * 512 + h2 * DE : qi * 512 + h2 * DE + D], rc
                            )
                        yns.append(yn)

                    def tail_transpose(qi, yns=yns, yts=yts):
                        # DMA-transpose XBAR: yn [128q, (h2,d)=128] -> yt[:, qi, :]
                        # = [(h2,d)=128, 128q], skipping PE/psum entirely.  For
                        # the final query-pair use PE transposes instead: the
                        # serial end-tail can't afford DMA queue round-trips.
                        if qi == 0:
                            yts.append(ytp.tile([128, 2, 128], BF16, name="yt"))
                        yt = yts[-1]
                        if not last:
                            nc.sync.dma_start(
                                out=yt[:, qi, :],
                                in_=yns[qi].rearrange("p a d -> p (a d)"),
                                transpose=True,
                            )
                        else:
                            pyt = ps_o.tile([128, 128], BF16, name="pyt", tag="po")
                            for h2 in range(2):
                                nc.tensor.transpose(
                                    pyt[h2 * D : (h2 + 1) * D, :],
                                    yns[qi][:, h2, :],
                                    ident,
                                    tile_position=(0, h2 * D),
                                )
                            nc.vector.tensor_copy(yt[:, qi, :], pyt)

                    tail_transpose(0)
                    tail_transpose(1)

                def tail_outproj(qi, q0=q0, yts=yts):
                    qt = q0 + qi
                    osb = outp.tile([128, C], F32)
                    for ch in range(2):
                        po = ps_o.tile([128, 512], F32, name="po", tag="po")
                        for hp2 in range(PAIRS):
                            nc.tensor.matmul(
                                po,
                                lhsT=yts[hp2][:, qi, :],
                                rhs=wp_sb[:, hp2, ts(ch, 512)],
                                start=(hp2 == 0),
                                stop=(hp2 == PAIRS - 1),
                            )
                        nc.vector.tensor_copy(osb[:, ts(ch, 512)], po)
                        nc.sync.dma_start(
                            out=outd[qt * 128 : (qt + 1) * 128, ts(ch, 512)],
                            in_=osb[:, ts(ch, 512)],
                        )

                for qi in range(2):
                    work.append(("tail", lambda q=qi: tail_outproj(q)))

            # ---------------- main schedule ----------------
            dma_xT_group(0)
            # prologue: k for both pairs + v block 0 (enough for qp=0's first
            # groups); v blocks 1-2 flow through the work queue during hp0.
            # Alternate psum pools (ps_s is idle here) to avoid WAR serializing
            for pr in range(PAIRS):
                for rr in range(2):
                    emit_k_chunk(pr, rr, pool=(ps_s if rr else None))
            emit_v_chunk(0, 0, pool=ps_s)

            work = [("kv", lambda b=blk: emit_v_chunk(0, b)) for blk in (1, 2)]
            for G in range(4):
                if G < 3:
                    dma_xT_group(G + 1)
                    g1 = G + 1
                    for pr in range(PAIRS):
                        for rr in range(2):
                            work.append(
                                ("kv", lambda p=pr, r=2 * g1 + rr: emit_k_chunk(p, r))
                            )
                    for blk in range(3):
                        work.append(("kv", lambda g=g1, b=blk: emit_v_chunk(g, b)))
                for qp in (G, G + 4, G + 8):
                    emit_q_proj(qp)
                    emit_attention(qp, work, last=(G == 3 and qp == G + 8))
                # k/v for the next group must be emitted before its attention
                for kind, fn in [it for it in work if it[0] == "kv"]:
                    fn()
                work = [it for it in work if it[0] != "kv"]
            for _, fn in work:
                fn()

    nc.finalize()  # Bacc: runs compile pipeline (event-sem split, reg alloc)
    return nc


def _get_nc():
    if "nc" not in _CACHE:
        _CACHE["nc"] = _build()
    return _CACHE["nc"]


def _shard(inputs):
    bf = ml_dtypes.bfloat16
    x = np.asarray(inputs["x"], dtype=np.float32)
    Wq = np.asarray(inputs["Wq"], dtype=np.float32)
    Wk = np.asarray(inputs["Wk"], dtype=np.float32)
    Wv = np.asarray(inputs["Wv"], dtype=np.float32)
    Wp = np.asarray(inputs["Wp"], dtype=np.float32)
    bq = np.asarray(inputs["bq"], dtype=np.float32)
    bk = np.asarray(inputs["bk"], dtype=np.float32)
    bv = np.asarray(inputs["bv"], dtype=np.float32)

    in_maps = []
    for i in range(NCORE):
        b = i // 4
        j = i % 4
        hs = slice(j * CHS, (j + 1) * CHS)
        bqk = np.stack(
            [
                bq[hs].reshape(PAIRS, 128)[0],
                bq[hs].reshape(PAIRS, 128)[1],
                bk[hs].reshape(PAIRS, 128)[0],
                bk[hs].reshape(PAIRS, 128)[1],
            ],
            axis=1,
        )  # [128, 4]: (bq pr0, bq pr1, bk pr0, bk pr1)
        wqkv = np.concatenate([Wq[hs].T, Wk[hs].T, Wv[hs].T], axis=1)
        m = {
            "xT": np.ascontiguousarray(x[b].T).astype(bf),
            "wqkv": np.ascontiguousarray(wqkv).astype(bf),
            "wpT": np.ascontiguousarray(Wp[:, hs].T).astype(bf),
            "bqk": np.ascontiguousarray(bqk).astype(np.float32),
            "bv": np.ascontiguousarray(np.broadcast_to(bv[hs], (128, CHS))).astype(
                np.float32
            ),
        }
        in_maps.append(m)
    return in_maps


def _unshard(results, inputs):
    bp = np.asarray(inputs["bp"], dtype=np.float32)
    out = np.empty((B, T, C), dtype=np.float32)
    for b in range(B):
        acc = results[4 * b]["out"].astype(np.float32).copy()
        for j in range(1, 4):
            acc += results[4 * b + j]["out"]
        out[b] = acc + bp
    return out


def run(inputs, trace=False):
    nc = _get_nc()
    in_maps = _shard(inputs)
    res = run_bass_kernel_spmd(nc, in_maps, list(range(NCORE)), trace=trace)
    return _unshard(res.results, inputs), res


def kernel(**inputs):
    out, _ = run(inputs, trace=False)
    return out


# revision 34
# speedup vs baseline: 1.0275x; 1.0275x over previous
"""Trainium2 Bass kernel for CausalSelfAttention with block-repeated causal mask.

Problem: B=2, T=3072, C=1024, H=16 heads, d=64.
  q/k/v = x @ W{q,k,v}.T + b;  scores = q k^T / 8, masked by
  (i % 1024) >= (j % 1024) (tril(1024) tiled 3x3), softmax, y = attn @ v,
  out = y @ Wp.T + bp.

Sharding (8 cores): core i handles batch b = i//4 and heads 4*(i%4)..4*(i%4)+3
(data parallel on B, tensor parallel on heads).  Each core computes a partial
output projection (its 4 heads' contribution, no bias); the host sums the 4
partials per batch and adds bp (the TP all-reduce done at unshard time).

Device schedule (v2): query tile-pairs processed in order of increasing
block-row (ri1 = 1,3,5,7); k/v projections are computed incrementally for the
newly needed key-tile rows and interleaved as PE filler between score groups,
so the ACT engine (exp softmax, the co-bottleneck) is fed from ~10us onward
instead of idling behind an 80us projection prologue.  xT streams in per
group.

Per (query-pair, head-pair), key tiles go in groups of 2 into a [128,1024]
score psum laid out h2*512 + p*128 with p = (j0qi0, j0qi1, j1qi1, j1qi0),
so the fully-masked (j1,qi0) combination of diagonal groups is neither
scored nor exp'd (ACT reads a [128,2,384] view); the non-diagonal j1 is a
single n=256 matmul streaming qi1-then-qi0 through a reversed rhs AP.
Scores are K=64 matmuls (kT/qT in a [128=pair-channels, pair, T] bf16
layout, tile_position row 64*h2; consecutive matmuls alternate psum banks).
exp on ACT (scale=1/8 folded), diagonal tile masks multiplied on DVE,
attn@v lags one group behind exp and accumulates [q, (qi-bank, h2)] with a
ones column for rowsums, reciprocal-normalize on DVE, DMA-transpose XBAR
to [d, q] (PE transpose for the final pair's serial tail), output
projection per query tile.  Tails (transpose, outproj) are deferred through
the same work queue as the k/v filler chunks, one item per score group, so
the in-order PE never waits on the DVE normalize chain.
"""

import numpy as np
import ml_dtypes

import concourse.bass as bass
from concourse import bacc
import concourse.mybir as mybir
from concourse.bass import ts
from concourse.tile import TileContext
from concourse.bass_utils import run_bass_kernel_spmd
from concourse.masks import make_identity, make_upper_triangular

B, T, C, H = 2, 3072, 1024, 16
D = 64                  # head dim
NCORE = 8
HPC = 4                 # heads per core
PAIRS = 2               # head pairs per core
CHS = HPC * D           # 256 channels per core
NKT = T // 128          # 24 key tiles
RPB = 8                 # 128-tiles per 1024 mask block
DE = D + 1              # head value cols incl. ones column
NCI = C // 128          # 8 contraction tiles

BF16 = mybir.dt.bfloat16
F32 = mybir.dt.float32

_CACHE = {}


def _build():
    nc = bacc.Bacc()

    xTd = nc.dram_tensor("xT", [C, T], BF16, kind="ExternalInput")
    wqkvd = nc.dram_tensor("wqkv", [C, 3 * CHS], BF16, kind="ExternalInput")
    wpd = nc.dram_tensor("wpT", [CHS, C], BF16, kind="ExternalInput")
    bqkd = nc.dram_tensor("bqk", [128, 2 * PAIRS], F32, kind="ExternalInput")
    bvd = nc.dram_tensor("bv", [128, CHS], F32, kind="ExternalInput")
    outd = nc.dram_tensor("out", [T, C], F32, kind="ExternalOutput")

    with TileContext(nc) as tc:
        with (
            tc.tile_pool(name="const", bufs=1) as const,
            tc.tile_pool(name="exps", bufs=8) as expp,
            tc.tile_pool(name="ynorm", bufs=8) as ynp,
            tc.tile_pool(name="ytp", bufs=6) as ytp,
            tc.tile_pool(name="outp", bufs=3) as outp,
            tc.tile_pool(name="small", bufs=16) as smallp,
            tc.tile_pool(name="ps_s", bufs=2, space="PSUM") as ps_s,
            tc.tile_pool(name="ps_y", bufs=1, space="PSUM") as ps_y,
            tc.tile_pool(name="ps_o", bufs=2, space="PSUM") as ps_o,
        ):
            # ---------------- constants / weights into SBUF ----------------
            xT_sb = const.tile([128, NCI, T], BF16)
            xT_r = xTd[:, :].rearrange("(a p) (blk t) -> a p blk t", p=128, blk=3)

            def dma_xT_group(g):
                # tiles r in (2g, 2g+1) for all 3 blocks: 256 cols per block
                for a in range(NCI):
                    dst = xT_sb[:, a, :].rearrange("p (blk t) -> p blk t", blk=3)
                    nc.sync.dma_start(
                        out=dst[:, :, 2 * g * 128 : (2 * g + 2) * 128],
                        in_=xT_r[a][:, :, 2 * g * 128 : (2 * g + 2) * 128],
                    )

            # weights + biases ride the idle gpsimd DMA queue so the sync
            # queue's serial descriptor issue doesn't delay xT / outputs
            wqkv_sb = const.tile([128, NCI, 3 * CHS], BF16)
            wqkv_r = wqkvd[:, :].rearrange("(a p) c -> a p c", p=128)
            for a in range(NCI):  # wk section first: k-chunks gate the ramp
                nc.gpsimd.dma_start(
                    out=wqkv_sb[:, a, CHS : 2 * CHS], in_=wqkv_r[a][:, CHS : 2 * CHS]
                )
            for a in range(NCI):
                nc.gpsimd.dma_start(
                    out=wqkv_sb[:, a, 0:CHS], in_=wqkv_r[a][:, 0:CHS]
                )
                nc.gpsimd.dma_start(
                    out=wqkv_sb[:, a, 2 * CHS :], in_=wqkv_r[a][:, 2 * CHS :]
                )
            bqk_ld = const.tile([128, 2 * PAIRS], F32)
            bv_ld = const.tile([128, CHS], F32)
            nc.gpsimd.dma_start(out=bqk_ld, in_=bqkd[:, :])
            nc.gpsimd.dma_start(out=bv_ld, in_=bvd[:, :])
            wp_sb = const.tile([128, PAIRS, C], BF16)
            for a in range(PAIRS):
                nc.gpsimd.dma_start(
                    out=wp_sb[:, a, :],
                    in_=wpd[:, :].rearrange("(a p) c -> a p c", p=128)[a],
                )
            # DVE-local copies: consumers then never need a DMA sem wait
            bqk_sb = const.tile([128, 2 * PAIRS], F32)
            bv_sb = const.tile([128, CHS], F32)
            nc.vector.tensor_copy(bqk_sb, bqk_ld)
            nc.vector.tensor_copy(bv_sb, bv_ld)
            bv_r = bv_sb.rearrange("p (h e) -> p h e", e=D)

            ident = const.tile([128, 128], BF16)
            make_identity(nc, ident)
            # mask[k', q'] = 1 where q' >= k' (keep), else 0
            mask_sb = const.tile([128, 128], BF16)
            make_upper_triangular(nc, mask_sb, val=1.0, diag=True)

            # q/k in pair-channel layout [128 = (h_even 0:64 | h_odd 64:128), pair, t]
            qT_sb = const.tile([128, PAIRS, T], BF16)
            kT_sb = const.tile([128, PAIRS, T], BF16)
            v_sb = const.tile([128, NKT, HPC * DE], BF16)
            v_ones = v_sb.rearrange("p j (h e) -> p j h e", e=DE)[:, :, :, D : D + 1]
            nc.vector.memset(v_ones, 1.0)

            # ---------------- projection emitters ----------------
            def emit_k_chunk(pr, rr, pool=None):
                # kT pair pr for tile-row rr (=2g or 2g+1), all 3 blocks
                if pool is None:
                    pk = ps_o.tile([128, 512], F32, name="pk", tag="po")
                else:
                    pk = pool.tile([128, 512], F32, name="pscore", tag="sc")
                for ci in range(NCI):
                    rhs = xT_sb[:, ci, :].rearrange("p (blk t) -> p blk t", blk=3)
                    nc.tensor.matmul(
                        pk[:, 0:384],
                        lhsT=wqkv_sb[:, ci, CHS + pr * 128 : CHS + (pr + 1) * 128],
                        rhs=rhs[:, :, rr * 128 : (rr + 1) * 128],
                        start=(ci == 0),
                        stop=(ci == NCI - 1),
                    )
                dst = kT_sb[:, pr, :].rearrange("p (blk t) -> p blk t", blk=3)
                nc.vector.tensor_add(
                    dst[:, :, rr * 128 : (rr + 1) * 128],
                    pk[:, 0:384].rearrange("p (blk t) -> p blk t", blk=3),
                    bqk_sb[:, PAIRS + pr : PAIRS + pr + 1].to_broadcast((128, 3, 128)),
                )

            def emit_v_chunk(g, blk, pool=None):
                J0 = blk * RPB + 2 * g
                if pool is None:
                    pv = ps_o.tile([128, 512], F32, name="pv", tag="po")
                else:
                    pv = pool.tile([128, 512], F32, name="pscore", tag="sc")
                for sub in range(2):
                    tt = J0 + sub
                    for ci in range(NCI):
                        nc.tensor.matmul(
                            pv[:, ts(sub, 256)],
                            lhsT=xT_sb[:, ci, ts(tt, 128)],
                            rhs=wqkv_sb[:, ci, 2 * CHS : 3 * CHS],
                            start=(ci == 0),
                            stop=(ci == NCI - 1),
                        )
                for sub in range(2):
                    tt = J0 + sub
                    vt = v_sb[:, tt, :].rearrange("p (h e) -> p h e", e=DE)[:, :, 0:D]
                    pvr = pv[:, ts(sub, 256)].rearrange("p (h e) -> p h e", e=D)
                    nc.vector.tensor_add(vt, pvr, bv_r)

            def emit_q_proj(qp):
                c0 = 2 * qp * 128
                for pr in range(PAIRS):
                    pq = ps_o.tile([128, 512], F32, name="pq", tag="po")
                    for ci in range(NCI):
                        nc.tensor.matmul(
                            pq[:, 0:256],
                            lhsT=wqkv_sb[:, ci, ts(pr, 128)],
                            rhs=xT_sb[:, ci, c0 : c0 + 256],
                            start=(ci == 0),
                            stop=(ci == NCI - 1),
                        )
                    nc.vector.tensor_add(
                        qT_sb[:, pr, c0 : c0 + 256],
                        pq[:, 0:256],
                        bqk_sb[:, pr : pr + 1].to_broadcast((128, 256)),
                    )

            # ---------------- attention per query tile-pair ----------------
            def emit_attention(qp, work, last=False):
                ri0 = (2 * qp) % RPB
                ri1 = ri0 + 1
                q0 = 2 * qp
                allowed = [b * RPB + r for b in range(3) for r in range(ri1 + 1)]
                allowed_q = [
                    [j for j in allowed if j % RPB <= ri0],
                    allowed,
                ]
                groups = [allowed[i : i + 2] for i in range(0, len(allowed), 2)]

                it = 0
                yts = []
                for hp in range(PAIRS):
                    # [q, (qi-bank, h2)] psum: qi0 cols 0:130 (bank0), qi1
                    # cols 512:642 (bank1) so qi-inner attn@v alternates banks
                    py = ps_y.tile([128, 1024], F32, name="py", tag="py")

                    def emit_attnv(g, esb, py=py, allowed=allowed, allowed_q=allowed_q, hp=hp, ri1=ri1):
                        diag = g[1] % RPB == ri1
                        for j, J in enumerate(g):
                            for h2 in range(2):
                                hg = hp * 2 + h2
                                for qi in range(2):
                                    if j == 1 and qi == 0 and diag:
                                        continue
                                    p = 2 * j + (qi if j == 0 else 1 - qi)
                                    nc.tensor.matmul(
                                        py[:, qi * 512 + h2 * DE : qi * 512 + (h2 + 1) * DE],
                                        lhsT=esb[
                                            :,
                                            h2 * 512 + p * 128 : h2 * 512 + p * 128 + 128,
                                        ],
                                        rhs=v_sb[:, J, hg * DE : (hg + 1) * DE],
                                        start=(J == allowed[0] and h2 == 0),
                                        stop=(J == allowed_q[qi][-1]),
                                        skip_group_check=True,
                                    )

                    prev = None  # attn@v lags one group: never waits on exp
                    for gi, g in enumerate(groups):
                        diag = g[1] % RPB == ri1
                        pscore = ps_s.tile([128, 1024], F32, name="pscore", tag="sc")
                        # col layout per h2: p*128 with p = (j0qi0, j0qi1,
                        # j1qi1, j1qi0) so the diagonal groups' unscored
                        # (j1,qi0) block sits at the end of the exp view.
                        # j1 streams qi1-then-qi0 via a qi-reversed rhs AP.
                        rq = qT_sb[:, hp, q0 * 128 : q0 * 128 + 256]
                        rq_rev = rq.rearrange("p (a c) -> p a c", a=2)[:, ::-1, :]
                        for h2 in range(2):
                            nc.tensor.matmul(
                                pscore[:, h2 * 512 : h2 * 512 + 256],
                                lhsT=kT_sb[64 * h2 : 64 * h2 + 64, hp, ts(g[0], 128)],
                                rhs=rq[64 * h2 : 64 * h2 + 64, :],
                                start=True,
                                stop=True,
                                tile_position=(64 * h2, 0),
                                skip_group_check=True,
                            )
                        for h2 in range(2):
                            nc.tensor.matmul(
                                pscore[:, h2 * 512 + 256 : h2 * 512 + 512],
                                lhsT=kT_sb[64 * h2 : 64 * h2 + 64, hp, ts(g[1], 128)],
                                rhs=rq_rev[64 * h2 : 64 * h2 + 64, :, :],
                                start=False,
                                stop=True,
                                tile_position=(64 * h2, 0),
                                skip_group_check=True,
                            )
                        esb = expp.tile([128, 1024], BF16)
                        if diag:
                            nc.scalar.activation(
                                esb.rearrange("p (h c) -> p h c", h=2)[:, :, 0:384],
                                pscore.rearrange("p (h c) -> p h c", h=2)[:, :, 0:384],
                                mybir.ActivationFunctionType.Exp,
                                scale=0.125,
                            )
                            # mask the two diagonal tiles: (j0,qi0)=p0, (j1,qi1)=p2
                            for h2 in range(2):
                                for p in (0, 2):
                                    sl = esb[:, h2 * 512 + p * 128 : h2 * 512 + p * 128 + 128]
                                    nc.vector.tensor_mul(sl, sl, mask_sb)
                        else:
                            nc.scalar.activation(
                                esb, pscore, mybir.ActivationFunctionType.Exp, scale=0.125
                            )
                        it += 1
                        if work:
                            work.pop(0)[1]()
                        if last:
                            emit_attnv(g, esb)
                        else:
                            if prev is not None:
                                emit_attnv(*prev)
                            prev = (g, esb)
                    if not last:
                        emit_attnv(*prev)

                    # normalize on DVE immediately (frees py for the next
                    # head-pair without blocking PE); defer the PE transposes
                    yns = []
                    for qi in range(2):
                        yn = ynp.tile([128, 2, D], BF16, name="yn")
                        for h2 in range(2):
                            rc = smallp.tile([128, 1], F32)
                            nc.vector.reciprocal(
                                rc, py[:, qi * 512 + h2 * DE + D : qi * 512 + h2 * DE + DE]
                            )
                            nc.vector.tensor_scalar_mul(
                                yn[:, h2, :], py[:, qi * 512 + h2 * DE : qi * 512 + h2 * DE + D], rc
                            )
                        yns.append(yn)

                    def tail_transpose(qi, yns=yns, yts=yts):
                        # DMA-transpose XBAR: yn [128q, (h2,d)=128] -> yt[:, qi, :]
                        # = [(h2,d)=128, 128q], skipping PE/psum entirely.  For
                        # the final query-pair use PE transposes instead: the
                        # serial end-tail can't afford DMA queue round-trips.
                        if qi == 0:
                            yts.append(ytp.tile([128, 2, 128], BF16, name="yt"))
                        yt = yts[-1]
                        if not last:
                            nc.sync.dma_start(
                                out=yt[:, qi, :],
                                in_=yns[qi].rearrange("p a d -> p (a d)"),
                                transpose=True,
                            )
                        else:
                            pyt = ps_o.tile([128, 128], BF16, name="pyt", tag="po")
                            for h2 in range(2):
                                nc.tensor.transpose(
                                    pyt[h2 * D : (h2 + 1) * D, :],
                                    yns[qi][:, h2, :],
                                    ident,
                                    tile_position=(0, h2 * D),
                                )
                            nc.vector.tensor_copy(yt[:, qi, :], pyt)

                    work.append(("tail", lambda f=tail_transpose: f(0)))
                    work.append(("tail", lambda f=tail_transpose: f(1)))

                def tail_outproj(qi, q0=q0, yts=yts):
                    qt = q0 + qi
                    osb = outp.tile([128, C], F32)
                    for ch in range(2):
                        po = ps_o.tile([128, 512], F32, name="po", tag="po")
                        for hp2 in range(PAIRS):
                            nc.tensor.matmul(
                                po,
                                lhsT=yts[hp2][:, qi, :],
                                rhs=wp_sb[:, hp2, ts(ch, 512)],
                                start=(hp2 == 0),
                                stop=(hp2 == PAIRS - 1),
                            )
                        nc.vector.tensor_copy(osb[:, ts(ch, 512)], po)
                        nc.sync.dma_start(
                            out=outd[qt * 128 : (qt + 1) * 128, ts(ch, 512)],
                            in_=osb[:, ts(ch, 512)],
                        )

                for qi in range(2):
                    work.append(("tail", lambda q=qi: tail_outproj(q)))

            # ---------------- main schedule ----------------
            dma_xT_group(0)
            # prologue: k for both pairs + v block 0 (enough for qp=0's first
            # groups); v blocks 1-2 flow through the work queue during hp0.
            # Alternate psum pools (ps_s is idle here) to avoid WAR serializing
            for pr in range(PAIRS):
                for rr in range(2):
                    emit_k_chunk(pr, rr, pool=(ps_s if rr else None))
            emit_v_chunk(0, 0, pool=ps_s)

            work = [("kv", lambda b=blk: emit_v_chunk(0, b)) for blk in (1, 2)]
            for G in range(4):
                if G < 3:
                    dma_xT_group(G + 1)
                    g1 = G + 1
                    for pr in range(PAIRS):
                        for rr in range(2):
                            work.append(
                                ("kv", lambda p=pr, r=2 * g1 + rr: emit_k_chunk(p, r))
                            )
                    for blk in range(3):
                        work.append(("kv", lambda g=g1, b=blk: emit_v_chunk(g, b)))
                for qp in (G, G + 4, G + 8):
                    emit_q_proj(qp)
                    emit_attention(qp, work, last=(G == 3 and qp == G + 8))
                # k/v for the next group must be emitted before its attention
                for kind, fn in [it for it in work if it[0] == "kv"]:
                    fn()
                work = [it for it in work if it[0] != "kv"]
            for _, fn in work:
                fn()

    nc.finalize()  # Bacc: runs compile pipeline (event-sem split, reg alloc)
    return nc


def _get_nc():
    if "nc" not in _CACHE:
        _CACHE["nc"] = _build()
    return _CACHE["nc"]


def _shard(inputs):
    bf = ml_dtypes.bfloat16
    x = np.asarray(inputs["x"], dtype=np.float32)
    Wq = np.asarray(inputs["Wq"], dtype=np.float32)
    Wk = np.asarray(inputs["Wk"], dtype=np.float32)
    Wv = np.asarray(inputs["Wv"], dtype=np.float32)
    Wp = np.asarray(inputs["Wp"], dtype=np.float32)
    bq = np.asarray(inputs["bq"], dtype=np.float32)
    bk = np.asarray(inputs["bk"], dtype=np.float32)
    bv = np.asarray(inputs["bv"], dtype=np.float32)

    in_maps = []
    for i in range(NCORE):
        b = i // 4
        j = i % 4
        hs = slice(j * CHS, (j + 1) * CHS)
        bqk = np.stack(
            [
                bq[hs].reshape(PAIRS, 128)[0],
                bq[hs].reshape(PAIRS, 128)[1],
                bk[hs].reshape(PAIRS, 128)[0],
                bk[hs].reshape(PAIRS, 128)[1],
            ],
            axis=1,
        )  # [128, 4]: (bq pr0, bq pr1, bk pr0, bk pr1)
        wqkv = np.concatenate([Wq[hs].T, Wk[hs].T, Wv[hs].T], axis=1)
        m = {
            "xT": np.ascontiguousarray(x[b].T).astype(bf),
            "wqkv": np.ascontiguousarray(wqkv).astype(bf),
            "wpT": np.ascontiguousarray(Wp[:, hs].T).astype(bf),
            "bqk": np.ascontiguousarray(bqk).astype(np.float32),
            "bv": np.ascontiguousarray(np.broadcast_to(bv[hs], (128, CHS))).astype(
                np.float32
            ),
        }
        in_maps.append(m)
    return in_maps


def _unshard(results, inputs):
    bp = np.asarray(inputs["bp"], dtype=np.float32)
    out = np.empty((B, T, C), dtype=np.float32)
    for b in range(B):
        acc = results[4 * b]["out"].astype(np.float32).copy()
        for j in range(1, 4):
            acc += results[4 * b + j]["out"]
        out[b] = acc + bp
    return out


def run(inputs, trace=False):
    nc = _get_nc()
    in_maps = _shard(inputs)
    res = run_bass_kernel_spmd(nc, in_maps, list(range(NCORE)), trace=trace)
    return _unshard(res.results, inputs), res


def kernel(**inputs):
    out, _ = run(inputs, trace=False)
    return out


# revision 35
# speedup vs baseline: 1.0708x; 1.0421x over previous
"""Trainium2 Bass kernel for CausalSelfAttention with block-repeated causal mask.

Problem: B=2, T=3072, C=1024, H=16 heads, d=64.
  q/k/v = x @ W{q,k,v}.T + b;  scores = q k^T / 8, masked by
  (i % 1024) >= (j % 1024) (tril(1024) tiled 3x3), softmax, y = attn @ v,
  out = y @ Wp.T + bp.

Sharding (8 cores): core i handles batch b = i//4 and heads 4*(i%4)..4*(i%4)+3
(data parallel on B, tensor parallel on heads).  Each core computes a partial
output projection (its 4 heads' contribution, no bias); the host sums the 4
partials per batch and adds bp (the TP all-reduce done at unshard time).

Device schedule (v2): query tile-pairs processed in order of increasing
block-row (ri1 = 1,3,5,7); k/v projections are computed incrementally for the
newly needed key-tile rows and interleaved as PE filler between score groups,
so the ACT engine (exp softmax, the co-bottleneck) is fed from ~10us onward
instead of idling behind an 80us projection prologue.  xT streams in per
group.

Per (query-pair, head-pair), key tiles go in groups of 2 into a [128,1024]
score psum laid out h2*512 + p*128 with p = (j0qi0, j0qi1, j1qi1, j1qi0),
so the fully-masked (j1,qi0) combination of diagonal groups is neither
scored nor exp'd (ACT reads a [128,2,384] view); the non-diagonal j1 is a
single n=256 matmul streaming qi1-then-qi0 through a reversed rhs AP.
Scores are K=64 matmuls (kT/qT in a [128=pair-channels, pair, T] bf16
layout, tile_position row 64*h2; consecutive matmuls alternate psum banks).
exp on ACT (scale=1/8 folded), diagonal tile masks multiplied on DVE,
attn@v lags one group behind exp and accumulates [q, (qi-bank, h2)] with a
ones column for rowsums, reciprocal-normalize on DVE, DMA-transpose XBAR
to [d, q] (PE transpose for the final pair's serial tail), output
projection per query tile.  Tails (transpose, outproj) are deferred through
the same work queue as the k/v filler chunks, one item per score group, so
the in-order PE never waits on the DVE normalize chain.
"""

import numpy as np
import ml_dtypes

import concourse.bass as bass
from concourse import bacc
import concourse.mybir as mybir
from concourse.bass import ts
from concourse.tile import TileContext
from concourse.bass_utils import run_bass_kernel_spmd
from concourse.masks import make_identity, make_upper_triangular

B, T, C, H = 2, 3072, 1024, 16
D = 64                  # head dim
NCORE = 8
HPC = 4                 # heads per core
PAIRS = 2               # head pairs per core
CHS = HPC * D           # 256 channels per core
NKT = T // 128          # 24 key tiles
RPB = 8                 # 128-tiles per 1024 mask block
DE = D + 1              # head value cols incl. ones column
NCI = C // 128          # 8 contraction tiles

BF16 = mybir.dt.bfloat16
F32 = mybir.dt.float32

_CACHE = {}


def _build():
    nc = bacc.Bacc()

    xTd = nc.dram_tensor("xT", [C, T], BF16, kind="ExternalInput")
    wqkvd = nc.dram_tensor("wqkv", [C, 3 * CHS], BF16, kind="ExternalInput")
    wpd = nc.dram_tensor("wpT", [CHS, C], BF16, kind="ExternalInput")
    bqkd = nc.dram_tensor("bqk", [128, 2 * PAIRS], F32, kind="ExternalInput")
    bvd = nc.dram_tensor("bv", [128, CHS], F32, kind="ExternalInput")
    outd = nc.dram_tensor("out", [T, C], F32, kind="ExternalOutput")

    with TileContext(nc) as tc:
        with (
            tc.tile_pool(name="const", bufs=1) as const,
            tc.tile_pool(name="exps", bufs=8) as expp,
            tc.tile_pool(name="ynorm", bufs=8) as ynp,
            tc.tile_pool(name="ytp", bufs=6) as ytp,
            tc.tile_pool(name="outp", bufs=3) as outp,
            tc.tile_pool(name="small", bufs=16) as smallp,
            tc.tile_pool(name="ps_s", bufs=2, space="PSUM") as ps_s,
            tc.tile_pool(name="ps_y", bufs=1, space="PSUM") as ps_y,
            tc.tile_pool(name="ps_o", bufs=2, space="PSUM") as ps_o,
        ):
            # ---------------- constants / weights into SBUF ----------------
            xT_sb = const.tile([128, NCI, T], BF16)
            xT_r = xTd[:, :].rearrange("(a p) (blk t) -> a p blk t", p=128, blk=3)

            def dma_xT_group(g):
                # tiles r in (2g, 2g+1) for all 3 blocks: 256 cols per block
                for a in range(NCI):
                    dst = xT_sb[:, a, :].rearrange("p (blk t) -> p blk t", blk=3)
                    nc.sync.dma_start(
                        out=dst[:, :, 2 * g * 128 : (2 * g + 2) * 128],
                        in_=xT_r[a][:, :, 2 * g * 128 : (2 * g + 2) * 128],
                    )

            # weights + biases ride the idle gpsimd DMA queue so the sync
            # queue's serial descriptor issue doesn't delay xT / outputs
            wqkv_sb = const.tile([128, NCI, 3 * CHS], BF16)
            for a in range(NCI):
                nc.gpsimd.dma_start(
                    out=wqkv_sb[:, a, :],
                    in_=wqkvd[:, :].rearrange("(a p) c -> a p c", p=128)[a],
                )
            bqk_ld = const.tile([128, 2 * PAIRS], F32)
            bv_ld = const.tile([128, CHS], F32)
            nc.gpsimd.dma_start(out=bqk_ld, in_=bqkd[:, :])
            nc.gpsimd.dma_start(out=bv_ld, in_=bvd[:, :])
            wp_sb = const.tile([128, PAIRS, C], BF16)
            for a in range(PAIRS):
                nc.gpsimd.dma_start(
                    out=wp_sb[:, a, :],
                    in_=wpd[:, :].rearrange("(a p) c -> a p c", p=128)[a],
                )
            # DVE-local copies: consumers then never need a DMA sem wait
            bqk_sb = const.tile([128, 2 * PAIRS], F32)
            bv_sb = const.tile([128, CHS], F32)
            nc.vector.tensor_copy(bqk_sb, bqk_ld)
            nc.vector.tensor_copy(bv_sb, bv_ld)
            bv_r = bv_sb.rearrange("p (h e) -> p h e", e=D)

            ident = const.tile([128, 128], BF16)
            make_identity(nc, ident)
            # mask[k', q'] = 1 where q' >= k' (keep), else 0
            mask_sb = const.tile([128, 128], BF16)
            make_upper_triangular(nc, mask_sb, val=1.0, diag=True)

            # q/k in pair-channel layout [128 = (h_even 0:64 | h_odd 64:128), pair, t]
            qT_sb = const.tile([128, PAIRS, T], BF16)
            kT_sb = const.tile([128, PAIRS, T], BF16)
            v_sb = const.tile([128, NKT, HPC * DE], BF16)
            v_ones = v_sb.rearrange("p j (h e) -> p j h e", e=DE)[:, :, :, D : D + 1]
            nc.vector.memset(v_ones, 1.0)

            # ---------------- projection emitters ----------------
            def emit_k_chunk(pr, rr, pool=None):
                # kT pair pr for tile-row rr (=2g or 2g+1), all 3 blocks
                if pool is None:
                    pk = ps_o.tile([128, 512], F32, name="pk", tag="po")
                else:
                    pk = pool.tile([128, 512], F32, name="pscore", tag="sc")
                for ci in range(NCI):
                    rhs = xT_sb[:, ci, :].rearrange("p (blk t) -> p blk t", blk=3)
                    nc.tensor.matmul(
                        pk[:, 0:384],
                        lhsT=wqkv_sb[:, ci, CHS + pr * 128 : CHS + (pr + 1) * 128],
                        rhs=rhs[:, :, rr * 128 : (rr + 1) * 128],
                        start=(ci == 0),
                        stop=(ci == NCI - 1),
                    )
                dst = kT_sb[:, pr, :].rearrange("p (blk t) -> p blk t", blk=3)
                nc.vector.tensor_add(
                    dst[:, :, rr * 128 : (rr + 1) * 128],
                    pk[:, 0:384].rearrange("p (blk t) -> p blk t", blk=3),
                    bqk_sb[:, PAIRS + pr : PAIRS + pr + 1].to_broadcast((128, 3, 128)),
                )

            def emit_v_chunk(g, blk, pool=None):
                J0 = blk * RPB + 2 * g
                if pool is None:
                    pv = ps_o.tile([128, 512], F32, name="pv", tag="po")
                else:
                    pv = pool.tile([128, 512], F32, name="pscore", tag="sc")
                for sub in range(2):
                    tt = J0 + sub
                    for ci in range(NCI):
                        nc.tensor.matmul(
                            pv[:, ts(sub, 256)],
                            lhsT=xT_sb[:, ci, ts(tt, 128)],
                            rhs=wqkv_sb[:, ci, 2 * CHS : 3 * CHS],
                            start=(ci == 0),
                            stop=(ci == NCI - 1),
                        )
                for sub in range(2):
                    tt = J0 + sub
                    vt = v_sb[:, tt, :].rearrange("p (h e) -> p h e", e=DE)[:, :, 0:D]
                    pvr = pv[:, ts(sub, 256)].rearrange("p (h e) -> p h e", e=D)
                    nc.vector.tensor_add(vt, pvr, bv_r)

            def emit_q_proj(qp):
                c0 = 2 * qp * 128
                for pr in range(PAIRS):
                    pq = ps_o.tile([128, 512], F32, name="pq", tag="po")
                    for ci in range(NCI):
                        nc.tensor.matmul(
                            pq[:, 0:256],
                            lhsT=wqkv_sb[:, ci, ts(pr, 128)],
                            rhs=xT_sb[:, ci, c0 : c0 + 256],
                            start=(ci == 0),
                            stop=(ci == NCI - 1),
                        )
                    nc.vector.tensor_add(
                        qT_sb[:, pr, c0 : c0 + 256],
                        pq[:, 0:256],
                        bqk_sb[:, pr : pr + 1].to_broadcast((128, 256)),
                    )

            # ---------------- attention per query tile-pair ----------------
            def emit_attention(qp, work, last=False):
                ri0 = (2 * qp) % RPB
                ri1 = ri0 + 1
                q0 = 2 * qp
                allowed = [b * RPB + r for b in range(3) for r in range(ri1 + 1)]
                allowed_q = [
                    [j for j in allowed if j % RPB <= ri0],
                    allowed,
                ]
                groups = [allowed[i : i + 2] for i in range(0, len(allowed), 2)]

                it = 0
                yts = []
                for hp in range(PAIRS):
                    # [q, (qi-bank, h2)] psum: qi0 cols 0:130 (bank0), qi1
                    # cols 512:642 (bank1) so qi-inner attn@v alternates banks
                    py = ps_y.tile([128, 1024], F32, name="py", tag="py")

                    def emit_attnv(g, esb, py=py, allowed=allowed, allowed_q=allowed_q, hp=hp, ri1=ri1):
                        diag = g[1] % RPB == ri1
                        for j, J in enumerate(g):
                            for h2 in range(2):
                                hg = hp * 2 + h2
                                for qi in range(2):
                                    if j == 1 and qi == 0 and diag:
                                        continue
                                    p = 2 * j + (qi if j == 0 else 1 - qi)
                                    nc.tensor.matmul(
                                        py[:, qi * 512 + h2 * DE : qi * 512 + (h2 + 1) * DE],
                                        lhsT=esb[
                                            :,
                                            h2 * 512 + p * 128 : h2 * 512 + p * 128 + 128,
                                        ],
                                        rhs=v_sb[:, J, hg * DE : (hg + 1) * DE],
                                        start=(J == allowed[0] and h2 == 0),
                                        stop=(J == allowed_q[qi][-1]),
                                        skip_group_check=True,
                                    )

                    prev = None  # attn@v lags one group: never waits on exp
                    for gi, g in enumerate(groups):
                        diag = g[1] % RPB == ri1
                        pscore = ps_s.tile([128, 1024], F32, name="pscore", tag="sc")
                        # col layout per h2: p*128 with p = (j0qi0, j0qi1,
                        # j1qi1, j1qi0) so the diagonal groups' unscored
                        # (j1,qi0) block sits at the end of the exp view.
                        # j1 streams qi1-then-qi0 via a qi-reversed rhs AP.
                        rq = qT_sb[:, hp, q0 * 128 : q0 * 128 + 256]
                        rq_rev = rq.rearrange("p (a c) -> p a c", a=2)[:, ::-1, :]
                        for h2 in range(2):
                            nc.tensor.matmul(
                                pscore[:, h2 * 512 : h2 * 512 + 256],
                                lhsT=kT_sb[64 * h2 : 64 * h2 + 64, hp, ts(g[0], 128)],
                                rhs=rq[64 * h2 : 64 * h2 + 64, :],
                                start=True,
                                stop=True,
                                tile_position=(64 * h2, 0),
                                skip_group_check=True,
                            )
                        for h2 in range(2):
                            nc.tensor.matmul(
                                pscore[:, h2 * 512 + 256 : h2 * 512 + 512],
                                lhsT=kT_sb[64 * h2 : 64 * h2 + 64, hp, ts(g[1], 128)],
                                rhs=rq_rev[64 * h2 : 64 * h2 + 64, :, :],
                                start=False,
                                stop=True,
                                tile_position=(64 * h2, 0),
                                skip_group_check=True,
                            )
                        esb = expp.tile([128, 1024], BF16)
                        if diag:
                            nc.scalar.activation(
                                esb.rearrange("p (h c) -> p h c", h=2)[:, :, 0:384],
                                pscore.rearrange("p (h c) -> p h c", h=2)[:, :, 0:384],
                                mybir.ActivationFunctionType.Exp,
                                scale=0.125,
                            )
                            # mask the two diagonal tiles: (j0,qi0)=p0, (j1,qi1)=p2
                            for h2 in range(2):
                                for p in (0, 2):
                                    sl = esb[:, h2 * 512 + p * 128 : h2 * 512 + p * 128 + 128]
                                    nc.vector.tensor_mul(sl, sl, mask_sb)
                        else:
                            nc.scalar.activation(
                                esb, pscore, mybir.ActivationFunctionType.Exp, scale=0.125
                            )
                        it += 1
                        if work:
                            work.pop(0)[1]()
                        if last:
                            emit_attnv(g, esb)
                        else:
                            if prev is not None:
                                emit_attnv(*prev)
                            prev = (g, esb)
                    if not last:
                        emit_attnv(*prev)

                    # normalize on DVE immediately (frees py for the next
                    # head-pair without blocking PE); defer the PE transposes
                    yns = []
                    for qi in range(2):
                        yn = ynp.tile([128, 2, D], BF16, name="yn")
                        for h2 in range(2):
                            rc = smallp.tile([128, 1], F32)
                            nc.vector.reciprocal(
                                rc, py[:, qi * 512 + h2 * DE + D : qi * 512 + h2 * DE + DE]
                            )
                            nc.vector.tensor_scalar_mul(
                                yn[:, h2, :], py[:, qi * 512 + h2 * DE : qi * 512 + h2 * DE + D], rc
                            )
                        yns.append(yn)

                    def tail_transpose(qi, yns=yns, yts=yts):
                        # DMA-transpose XBAR: yn [128q, (h2,d)=128] -> yt[:, qi, :]
                        # = [(h2,d)=128, 128q], skipping PE/psum entirely.  For
                        # the final query-pair use PE transposes instead: the
                        # serial end-tail can't afford DMA queue round-trips.
                        if qi == 0:
                            yts.append(ytp.tile([128, 2, 128], BF16, name="yt"))
                        yt = yts[-1]
                        if not last:
                            nc.sync.dma_start(
                                out=yt[:, qi, :],
                                in_=yns[qi].rearrange("p a d -> p (a d)"),
                                transpose=True,
                            )
                        else:
                            pyt = ps_o.tile([128, 128], BF16, name="pyt", tag="po")
                            for h2 in range(2):
                                nc.tensor.transpose(
                                    pyt[h2 * D : (h2 + 1) * D, :],
                                    yns[qi][:, h2, :],
                                    ident,
                                    tile_position=(0, h2 * D),
                                )
                            nc.vector.tensor_copy(yt[:, qi, :], pyt)

                    work.append(("tail", lambda f=tail_transpose: f(0)))
                    work.append(("tail", lambda f=tail_transpose: f(1)))

                def tail_outproj(qi, q0=q0, yts=yts):
                    qt = q0 + qi
                    osb = outp.tile([128, C], F32)
                    for ch in range(2):
                        po = ps_o.tile([128, 512], F32, name="po", tag="po")
                        for hp2 in range(PAIRS):
                            nc.tensor.matmul(
                                po,
                                lhsT=yts[hp2][:, qi, :],
                                rhs=wp_sb[:, hp2, ts(ch, 512)],
                                start=(hp2 == 0),
                                stop=(hp2 == PAIRS - 1),
                            )
                        nc.vector.tensor_copy(osb[:, ts(ch, 512)], po)
                        nc.sync.dma_start(
                            out=outd[qt * 128 : (qt + 1) * 128, ts(ch, 512)],
                            in_=osb[:, ts(ch, 512)],
                        )

                for qi in range(2):
                    work.append(("tail", lambda q=qi: tail_outproj(q)))

            # ---------------- main schedule ----------------
            dma_xT_group(0)
            # prologue: k for both pairs + v block 0 (enough for qp=0's first
            # groups); v blocks 1-2 flow through the work queue during hp0.
            # Alternate psum pools (ps_s is idle here) to avoid WAR serializing
            for pr in range(PAIRS):
                for rr in range(2):
                    emit_k_chunk(pr, rr, pool=(ps_s if rr else None))
            emit_v_chunk(0, 0, pool=ps_s)

            work = [("kv", lambda b=blk: emit_v_chunk(0, b)) for blk in (1, 2)]
            for G in range(4):
                if G < 3:
                    dma_xT_group(G + 1)
                    g1 = G + 1
                    for pr in range(PAIRS):
                        for rr in range(2):
                            work.append(
                                ("kv", lambda p=pr, r=2 * g1 + rr: emit_k_chunk(p, r))
                            )
                    for blk in range(3):
                        work.append(("kv", lambda g=g1, b=blk: emit_v_chunk(g, b)))
                for qp in (G, G + 4, G + 8):
                    emit_q_proj(qp)
                    emit_attention(qp, work, last=(G == 3 and qp == G + 8))
                # k/v for the next group must be emitted before its attention
                for kind, fn in [it for it in work if it[0] == "kv"]:
                    fn()
                work = [it for it in work if it[0] != "kv"]
            for _, fn in work:
                fn()

    nc.finalize()  # Bacc: runs compile pipeline (event-sem split, reg alloc)
    return nc


def _get_nc():
    if "nc" not in _CACHE:
        _CACHE["nc"] = _build()
    return _CACHE["nc"]


def _shard(inputs):
    bf = ml_dtypes.bfloat16
    x = np.asarray(inputs["x"], dtype=np.float32)
    Wq = np.asarray(inputs["Wq"], dtype=np.float32)
    Wk = np.asarray(inputs["Wk"], dtype=np.float32)
    Wv = np.asarray(inputs["Wv"], dtype=np.float32)
    Wp = np.asarray(inputs["Wp"], dtype=np.float32)
    bq = np.asarray(inputs["bq"], dtype=np.float32)
    bk = np.asarray(inputs["bk"], dtype=np.float32)
    bv = np.asarray(inputs["bv"], dtype=np.float32)

    in_maps = []
    for i in range(NCORE):
        b = i // 4
        j = i % 4
        hs = slice(j * CHS, (j + 1) * CHS)
        bqk = np.stack(
            [
                bq[hs].reshape(PAIRS, 128)[0],
                bq[hs].reshape(PAIRS, 128)[1],
                bk[hs].reshape(PAIRS, 128)[0],
                bk[hs].reshape(PAIRS, 128)[1],
            ],
            axis=1,
        )  # [128, 4]: (bq pr0, bq pr1, bk pr0, bk pr1)
        wqkv = np.concatenate([Wq[hs].T, Wk[hs].T, Wv[hs].T], axis=1)
        m = {
            "xT": np.ascontiguousarray(x[b].T).astype(bf),
            "wqkv": np.ascontiguousarray(wqkv).astype(bf),
            "wpT": np.ascontiguousarray(Wp[:, hs].T).astype(bf),
            "bqk": np.ascontiguousarray(bqk).astype(np.float32),
            "bv": np.ascontiguousarray(np.broadcast_to(bv[hs], (128, CHS))).astype(
                np.float32
            ),
        }
        in_maps.append(m)
    return in_maps


def _unshard(results, inputs):
    bp = np.asarray(inputs["bp"], dtype=np.float32)
    out = np.empty((B, T, C), dtype=np.float32)
    for b in range(B):
        acc = results[4 * b]["out"].astype(np.float32).copy()
        for j in range(1, 4):
            acc += results[4 * b + j]["out"]
        out[b] = acc + bp
    return out


def run(inputs, trace=False):
    nc = _get_nc()
    in_maps = _shard(inputs)
    res = run_bass_kernel_spmd(nc, in_maps, list(range(NCORE)), trace=trace)
    return _unshard(res.results, inputs), res


def kernel(**inputs):
    out, _ = run(inputs, trace=False)
    return out
